# revision 13
# baseline (speedup 1.0000x reference)
"""Hexagonal conv2d (HConv2D) Trainium2 kernel.

Math (verified vs the jax reference):
  out[n, 2i,   w, f] = relu(b + a[2i] + bb[2i+1] + c[2i+2])        (w-aligned)
  out[n, 2i+1, w, f] = relu(b + a'[2i+1][w] + c[2i+2][w-1])
with per-input-row 1D convs over Cin=128 -> F=256:
  a[r][w]  = k01.x[r,w]   + k02.x[r,w+1]
  a'[r][w] = k01.x[r,w-1] + k02.x[r,w]
  bb[r][w] = k10.x[r,w-1] + k11.x[r,w] + k12.x[r,w+1]
  c[r][w]  = k21.x[r,w]   + k22.x[r,w+1]
where krc = kernel[r, c] : [Cin, F].  KEY: c[2i+2] is shared between the
even and odd output rows (odd reads it shifted by one column), so it is
computed ONCE on the PE and added into both outputs on the Vector
engine: 9 big tap-matmuls per output-row-pair instead of 11 (18% PE cut).
The odd-row w=0 seam values (c[-1] = k22.x[2i+2, 0]) are computed up
front by four tiny N=64 matmuls whose x-column source rides inside the
weight DMA (spare tap slot 7), so no mid-stream op ever waits on them.

Distribution: data-parallel over batch (16 -> 8 cores x 2 images). Host
transposes x to [n, c, h, w] (c on partitions = contraction dim), pads
h/w with zeros, casts to bf16.  All data DMAs ride the sync engine's
HW-DGE ring (sync has no compute, so a trigger blocking on a congested
ring never stalls an engine that has real work; input triggers on the
ACT engine were observed to block its copy/ReLU stream mid-kernel).
The weight tensor is pre-split per f-chunk so the first matmul group
only waits for half of it.  A chain of N=128 dummy matmuls warms the PE
HAM clock-gate (1.2 -> 2.4 GHz needs ~3.4us of sustained PE-busy)
during the input-DMA wait.

PSUM tiles are [Fchunk=128, 4 rows, 128 w] (one bank, N=512; TRN2 PSUM
accumulation is fp32-only, so N=1024-bf16 is not available).  Per group
c is staged to SBUF by a SINGLE-writer copy (a second writer to the
same tile, even a disjoint slice, is serialized by the dependency
tracker into the ps_c-reuse chain); the odd parity adds the seam via a
tiny extra DVE op (odd emitted first - its psum closes 5 matmuls
earlier); one fused bias+ReLU on ScalarE emits bf16 and one DMA
per group writes a (n, hb, f, parity, row, w) bf16 DRAM layout
(per-partition contiguous 2 KB packets, half the bytes and packets of
fp32 NHWC).  The last two groups split ReLU+DMA by parity to shorten
the serial drain after the final matmul.  Host reassembles NHWC fp32.

Input arrives as 16-row chunks so the input stream holds twice the
DMA-ring round-robin share vs the queued output DMAs (input used to
land just-in-time at chunk boundaries, stalling the PE ~1us each).
Measured: ~145.8-147.1us on HW (baseline 147.5-148.6us), rel err
2.9e-3, 99% tensor-engine occupancy in the main phase; the remainder
is the bf16 PE feed floor (~130us: 294,912 moving columns at 1
col/cycle @ 2.4GHz + ~5ns/matmul issue) plus ~12us of fixed runtime
startup/teardown (engine preambles + ~9.8us semaphore-teardown
epilogue).
"""

import numpy as np
import ml_dtypes

import concourse.bacc as bacc
import concourse.bass as bass
import concourse.mybir as mybir
import concourse.tile as tile
from concourse.bass_utils import run_bass_kernel_spmd

N_CORES = 8
NPC = 2            # images per core
H = W = 128
C = 128            # input channels
F = 256            # filters
HP, WP = H + 1, W + 2
HB = 4             # out-row-pairs per psum tile (4 pairs -> N=512)
NHB = (H // 2) // HB

# tap weight order: kernel[r][c] for these (r, c)
TAP_RC = [(0, 1), (0, 2), (1, 0), (1, 1), (1, 2), (2, 1), (2, 2)]

BF16 = mybir.dt.bfloat16
F32 = mybir.dt.float32


def fsl_(fj):
    return slice(fj * 128, (fj + 1) * 128)


def _build():
    nc = bacc.Bacc(
        "TRN2", target_bir_lowering=False, debug=False, num_devices=N_CORES
    )
    xt = nc.dram_tensor("xt", (NPC, C, HP, WP), BF16, kind="ExternalInput").ap()
    # weights pre-split per f-chunk: (C, fj, tap, 128); tap slot 7 of half j
    # carries image j's x[2p+2, w=0] column (seam source), so the seam data
    # rides the weight DMA for free
    wt = nc.dram_tensor("wt", (C, 2, 8, F // 2), BF16, kind="ExternalInput").ap()
    bs = nc.dram_tensor("bs", (F // 2, 2), F32, kind="ExternalInput").ap()
    ot = nc.dram_tensor(
        "ot", (NPC, NHB, F, 2, HB, W), BF16, kind="ExternalOutput"
    ).ap()

    with tile.TileContext(nc) as tc:
        with (
            tc.tile_pool(name="const", bufs=1) as const,
            tc.tile_pool(name="xpool", bufs=1) as xpool,
            tc.tile_pool(name="psum", bufs=2, space="PSUM") as psum,
            tc.tile_pool(name="osb", bufs=4) as osb,
        ):
            xs = [
                xpool.tile([C, HP, WP], BF16, name=f"xs{n}", tag=f"xs{n}")
                for n in range(NPC)
            ]
            # ALL data DMAs ride the sync engine's ring.  The 16 HW DMA
            # engines are SHARED between queues (~300GB/s aggregate + ~1.5us
            # trigger->first-packet latency), so a second queue does not
            # speed up the startup prefix -- shrinking the prefix does.
            # Order: [fj0 taps 5,6,7 (96KB: all that the seam00 matmul and
            # group 1's ps_c taps need), input rows 0:9, fj0 taps 0:5, fj1
            # half, bias, input chunks].  Group 1's first matmuls start
            # ~9.4us instead of ~10.7 (full-prefix serialization).
            w_sb = const.tile([C, 2, 8, F // 2], BF16, name="w_sb")
            nc.sync.dma_start(out=w_sb[:, 0, 5:8], in_=wt[:, 0, 5:8])
            nc.sync.dma_start(out=xs[0][:, 0:9, :], in_=xt[0, :, 0:9, :])
            nc.sync.dma_start(out=w_sb[:, 0, 0:5], in_=wt[:, 0, 0:5])
            nc.sync.dma_start(out=w_sb[:, 1], in_=wt[:, 1])
            b_sb = const.tile([F // 2, 2], F32, name="b_sb")
            nc.sync.dma_start(out=b_sb[:], in_=bs[:])
            # 16-row chunks: finer-grained arrival and twice the ring
            # round-robin share for input vs the queued output DMAs
            for n in range(NPC):
                bounds = (
                    [9, 25, 41, 57, 73, 89, 105, 121, 129] if n == 0
                    else [0, 17, 33, 49, 65, 81, 97, 113, 129]
                )
                for h0, h1 in zip(bounds[:-1], bounds[1:]):
                    nc.sync.dma_start(
                        out=xs[n][:, h0:h1, :], in_=xt[n, :, h0:h1, :]
                    )

            # Warm the PE HAM clock-gate during the input-DMA wait: HAM needs
            # ~3.4us of sustained PE-busy to lift the 1.2->2.4GHz throttle.
            # The memset rides GPSIMD (its program starts earliest and it is
            # otherwise idle), so the warmup chain starts ~6.9us instead of
            # waiting for the vector engine (which now carries the weight DMA
            # triggers).  With weights parallel to input, real data is ready
            # ~8.6us; 20 cold (1.2GHz) N=128 matmuls cover ~2.1us.
            warm_sb = const.tile([128, 512], BF16, name="warm_sb")
            nc.gpsimd.memset(warm_sb[:], 0.0)
            ps_w = psum.tile([128, 512], F32, name="ps_w", tag="ps_c")
            NWARM = 21
            for i in range(NWARM):
                nc.tensor.matmul(
                    ps_w[:, 0:128], warm_sb[:, 0:128], warm_sb[:, 0:128],
                    start=(i == 0), stop=(i == NWARM - 1))

            add = mybir.AluOpType.add
            # per-(image, fchunk) seam planes: seam[p] = k22 . x[2p+2, w=0]
            # (the c[-1] column every odd row needs at w=0).  The x column
            # rides in w_sb[:, n, 7] so seams are tiny N=64 matmuls.  Only
            # the (0,0) seam runs before group 1 (gated on wt half 0); the
            # other three need wt half 1, which lands mid-group-1, so they
            # are emitted AFTER group 1's matmuls -- the PE queue is
            # in-order, and a seam parked ahead of group 1 waiting on wt1
            # stalls the whole stream (HAM re-throttles, ~5us of 1.2GHz
            # matmuls follow).
            seam_sb = [
                [
                    const.tile([128, 64, 1], F32, name=f"seam{n}{fj}")
                    for fj in range(2)
                ]
                for n in range(NPC)
            ]

            def emit_seam(n, fj):
                sp = psum.tile([128, 64, 1], F32, name="sps", tag="ps_c")
                nc.tensor.matmul(
                    sp[:], w_sb[:, fj, 6], w_sb[:, n, 7, 0:64],
                    start=True, stop=True)
                nc.vector.tensor_copy(seam_sb[n][fj][:], sp[:])

            emit_seam(0, 0)
            ngrp = NPC * NHB * 2
            gi = 0
            for n in range(NPC):
                for hb in range(NHB):
                    r0 = 2 * HB * hb
                    rE = slice(r0, r0 + 2 * HB - 1, 2)        # rows 2i
                    rO = slice(r0 + 1, r0 + 2 * HB, 2)        # rows 2i+1
                    rC = slice(r0 + 2, r0 + 2 * HB + 1, 2)    # rows 2i+2
                    for fj in range(2):
                        gi += 1
                        last2 = gi > ngrp - 2
                        if gi == 2:
                            # remaining seams, now that wt half 1 has landed
                            # (group 1's matmuls covered the wait)
                            for sn, sfj in ((0, 1), (1, 0), (1, 1)):
                                emit_seam(sn, sfj)

                        # Three PSUM accumulation groups, matmuls interleaved
                        # weight-major so identical stationary weights are
                        # back-to-back and ps_c finishes early (DVE pipeline).
                        # c[2i+2][w] = k21.x[w] + k22.x[w+1], w = 0..127
                        ps_c = psum.tile([128, HB, W], F32, name="ps_c", tag="ps_c")
                        # even: a[2i] + bb[2i+1]
                        ps_e = psum.tile(
                            [128, HB, W], F32, name="ps_e", tag="ps_e", bufs=3
                        )
                        # odd: a'[2i+1]; w=0 seam adds c[-1] = k22.x[0]
                        ps_o = psum.tile(
                            [128, HB, W], F32, name="ps_o", tag="ps_o", bufs=3
                        )
                        mm = nc.tensor.matmul
                        wv = w_sb[:, fj]
                        mm(ps_c[:], wv[:, 5], xs[n][:, rC, 1:129],
                           start=True, stop=False)
                        mm(ps_c[:], wv[:, 6], xs[n][:, rC, 2:130],
                           start=False, stop=True)
                        mm(ps_o[:], wv[:, 0], xs[n][:, rO, 0:128],
                           start=True, stop=False)
                        mm(ps_o[:], wv[:, 1], xs[n][:, rO, 1:129],
                           start=False, stop=True)
                        mm(ps_e[:], wv[:, 0], xs[n][:, rE, 1:129],
                           start=True, stop=False)
                        mm(ps_e[:], wv[:, 1], xs[n][:, rE, 2:130],
                           start=False, stop=False)
                        mm(ps_e[:], wv[:, 2], xs[n][:, rO, 0:128],
                           start=False, stop=False)
                        mm(ps_e[:], wv[:, 3], xs[n][:, rO, 1:129],
                           start=False, stop=False)
                        mm(ps_e[:], wv[:, 4], xs[n][:, rO, 2:130],
                           start=False, stop=True)

                        # DVE cannot read two PSUM operands in one op:
                        # stage c in SBUF, then add it into both parities.
                        # Alternate the copy engine to balance DVE vs ACT load
                        # (the last two groups pin it to ScalarE so the DVE
                        # adds start immediately after their matmuls).
                        # c staged with a SINGLE writer (a second writer to
                        # the same tile -- even a disjoint slice -- gets
                        # serialized by the dependency tracker and puts that
                        # engine's dispatch latency inside the ps_c-reuse
                        # chain).  Copy rides ScalarE 4 of 5 groups to
                        # balance DVE vs ACT (~1.66us each vs PE's 1.98us).
                        c_sb = osb.tile(
                            [128, HB, W], F32, name="c_sb", tag="cx", bufs=4
                        )
                        if last2 or (2 * hb + fj) % 5 != 0:
                            nc.scalar.copy(c_sb[:], ps_c[:])
                        else:
                            nc.vector.tensor_copy(c_sb[:], ps_c[:])
                        # both parities staged in one tile: [f, parity, row, w]
                        ob = osb.tile([128, 2, HB, W], F32, name="ob", tag="ob", bufs=5)
                        nc.vector.tensor_tensor(
                            ob[:, 1, :, 0:1], ps_o[:, :, 0:1],
                            seam_sb[n][fj][:, HB * hb : HB * hb + HB], op=add)
                        nc.vector.tensor_tensor(
                            ob[:, 1, :, 1:128], ps_o[:, :, 1:128],
                            c_sb[:, :, 0:127], op=add)
                        nc.vector.tensor_tensor(
                            ob[:, 0], ps_e[:], c_sb[:], op=add)

                        fo = osb.tile(
                            [128, 2, HB, W], BF16, name="fo", tag="fo", bufs=8
                        )
                        if last2:
                            # parity-split ReLU+DMA: the even half drains
                            # while the odd adds are still running.  For the
                            # very last group, the even half's bias+ReLU runs
                            # on GPSIMD (idle) in parallel with ACT's odd
                            # half, shortening the serial drain.
                            nc.scalar.activation(
                                fo[:, 1], ob[:, 1],
                                mybir.ActivationFunctionType.Relu,
                                bias=b_sb[:, fj : fj + 1],
                            )
                            nc.sync.dma_start(
                                out=ot[n, hb, fsl_(fj), 1, :, :],
                                in_=fo[:, 1],
                            )
                            nc.scalar.activation(
                                fo[:, 0], ob[:, 0],
                                mybir.ActivationFunctionType.Relu,
                                bias=b_sb[:, fj : fj + 1],
                            )
                            nc.sync.dma_start(
                                out=ot[n, hb, fsl_(fj), 0, :, :],
                                in_=fo[:, 0],
                            )
                        else:
                            nc.scalar.activation(
                                fo[:], ob[:],
                                mybir.ActivationFunctionType.Relu,
                                bias=b_sb[:, fj : fj + 1],
                            )
                            nc.sync.dma_start(
                                out=ot[n, hb, fsl_(fj), :, :, :], in_=fo[:]
                            )
    nc.compile()
    return nc


_NC_CACHE = None


def _get_nc():
    global _NC_CACHE
    if _NC_CACHE is None:
        _NC_CACHE = _build()
    return _NC_CACHE


def _prep_core_inputs(x_shard, wt_host, bs_host):
    xp = np.zeros((NPC, C, HP, WP), dtype=ml_dtypes.bfloat16)
    xp[:, :, :H, 1 : 1 + W] = x_shard.transpose(0, 3, 1, 2)
    wt = wt_host.copy()
    for n in range(NPC):
        # seam source: x[2p+2, w=0, c] for p=0..62 (p=63 is the zero pad row)
        wt[:, n, 7, 0:63] = (
            x_shard[n, 2:128:2, 0, :].T.astype(ml_dtypes.bfloat16)
        )
    return {"xt": xp, "wt": wt, "bs": bs_host}


def _unpack_out(ot_np):
    # ot: (NPC, NHB, F, 2, HB, W) bf16 -> (NPC, H, W, F) fp32
    # h = 8*hb + 2*i + par
    o = ot_np.astype(np.float32).transpose(0, 1, 4, 3, 5, 2)
    return o.reshape(NPC, H, W, F)


def _prep_host_weights(kernel, bias):
    # (C, 7, F) -> (C, fj, tap, 128), plus a spare tap slot 7 that each
    # core fills with its images' x[2p+2, w=0] columns (seam source)
    wt_host = np.zeros((C, 2, 8, F // 2), dtype=ml_dtypes.bfloat16)
    wt_host[:, :, 0:7, :] = (
        np.stack([kernel[r, c] for (r, c) in TAP_RC], axis=1)
        .reshape(C, 7, 2, F // 2)
        .transpose(0, 2, 1, 3)
    ).astype(ml_dtypes.bfloat16)
    bs_host = np.ascontiguousarray(
        bias.reshape(2, F // 2).T
    ).astype(np.float32)  # (128, 2): bs[f, j] = bias[j*128+f]
    return wt_host, bs_host


def kernel(x, kernel, bias):
    x = np.asarray(x, dtype=np.float32)
    kernel = np.asarray(kernel, dtype=np.float32)
    bias = np.asarray(bias, dtype=np.float32)

    wt_host, bs_host = _prep_host_weights(kernel, bias)

    nc = _get_nc()
    in_maps = [
        _prep_core_inputs(x[i * NPC : (i + 1) * NPC], wt_host, bs_host)
        for i in range(N_CORES)
    ]
    res = run_bass_kernel_spmd(nc, in_maps, list(range(N_CORES)))

    outs = [_unpack_out(res.results[i]["ot"]) for i in range(N_CORES)]
    return np.ascontiguousarray(np.concatenate(outs, axis=0))



# revision 16
# speedup vs baseline: 1.0031x; 1.0031x over previous
"""Hexagonal conv2d (HConv2D) Trainium2 kernel.

Math (verified vs the jax reference):
  out[n, 2i,   w, f] = relu(b + a[2i] + bb[2i+1] + c[2i+2])        (w-aligned)
  out[n, 2i+1, w, f] = relu(b + a'[2i+1][w] + c[2i+2][w-1])
with per-input-row 1D convs over Cin=128 -> F=256:
  a[r][w]  = k01.x[r,w]   + k02.x[r,w+1]
  a'[r][w] = k01.x[r,w-1] + k02.x[r,w]
  bb[r][w] = k10.x[r,w-1] + k11.x[r,w] + k12.x[r,w+1]
  c[r][w]  = k21.x[r,w]   + k22.x[r,w+1]
where krc = kernel[r, c] : [Cin, F].  KEY: c[2i+2] is shared between the
even and odd output rows (odd reads it shifted by one column), so it is
computed ONCE on the PE and added into both outputs on the Vector
engine: 9 big tap-matmuls per output-row-pair instead of 11 (18% PE cut).
The odd-row w=0 seam values (c[-1] = k22.x[2i+2, 0]) are computed up
front by four tiny N=64 matmuls whose x-column source rides inside the
weight DMA (spare tap slot 7), so no mid-stream op ever waits on them.

Distribution: data-parallel over batch (16 -> 8 cores x 2 images). Host
transposes x to [n, c, h, w] (c on partitions = contraction dim), pads
h/w with zeros, casts to bf16.  All data DMAs ride the sync engine's
HW-DGE ring (sync has no compute, so a trigger blocking on a congested
ring never stalls an engine that has real work; input triggers on the
ACT engine were observed to block its copy/ReLU stream mid-kernel).
The weight tensor is pre-split per f-chunk so the first matmul group
only waits for half of it.  A chain of N=128 dummy matmuls warms the PE
HAM clock-gate (1.2 -> 2.4 GHz needs ~3.4us of sustained PE-busy)
during the input-DMA wait.

PSUM tiles are [Fchunk=128, 4 rows, 128 w] (one bank, N=512; TRN2 PSUM
accumulation is fp32-only, so N=1024-bf16 is not available).  Per group
c is staged to SBUF by a SINGLE-writer copy (a second writer to the
same tile, even a disjoint slice, is serialized by the dependency
tracker into the ps_c-reuse chain); the odd parity adds the seam via a
tiny extra DVE op (odd emitted first - its psum closes 5 matmuls
earlier); one fused bias+ReLU on ScalarE emits bf16 and one DMA
per group writes a (n, hb, f, parity, row, w) bf16 DRAM layout
(per-partition contiguous 2 KB packets, half the bytes and packets of
fp32 NHWC).  The last two groups split ReLU+DMA by parity to shorten
the serial drain after the final matmul.  Host reassembles NHWC fp32.

Input arrives as 16-row chunks so the input stream holds twice the
DMA-ring round-robin share vs the queued output DMAs (input used to
land just-in-time at chunk boundaries, stalling the PE ~1us each).
Measured: ~145.8-147.1us on HW (baseline 147.5-148.6us), rel err
2.9e-3, 99% tensor-engine occupancy in the main phase; the remainder
is the bf16 PE feed floor (~130us: 294,912 moving columns at 1
col/cycle @ 2.4GHz + ~5ns/matmul issue) plus ~12us of fixed runtime
startup/teardown (engine preambles + ~9.8us semaphore-teardown
epilogue).
"""

import numpy as np
import ml_dtypes

import concourse.bacc as bacc
import concourse.bass as bass
import concourse.mybir as mybir
import concourse.tile as tile
from concourse.bass_utils import run_bass_kernel_spmd

N_CORES = 8
NPC = 2            # images per core
H = W = 128
C = 128            # input channels
F = 256            # filters
HP, WP = H + 1, W + 2
HB = 4             # out-row-pairs per psum tile (4 pairs -> N=512)
NHB = (H // 2) // HB

# tap weight order: kernel[r][c] for these (r, c)
TAP_RC = [(0, 1), (0, 2), (1, 0), (1, 1), (1, 2), (2, 1), (2, 2)]

BF16 = mybir.dt.bfloat16
F32 = mybir.dt.float32


def fsl_(fj):
    return slice(fj * 128, (fj + 1) * 128)


def _build():
    nc = bacc.Bacc(
        "TRN2", target_bir_lowering=False, debug=False, num_devices=N_CORES
    )
    xt = nc.dram_tensor("xt", (NPC, C, HP, WP), BF16, kind="ExternalInput").ap()
    # weights pre-split per f-chunk: (C, fj, tap, 128); tap slot 7 of half j
    # carries image j's x[2p+2, w=0] column (seam source), so the seam data
    # rides the weight DMA for free
    wt = nc.dram_tensor("wt", (C, 2, 8, F // 2), BF16, kind="ExternalInput").ap()
    bs = nc.dram_tensor("bs", (F // 2, 2), F32, kind="ExternalInput").ap()
    ot = nc.dram_tensor(
        "ot", (NPC, NHB, F, 2, HB, W), BF16, kind="ExternalOutput"
    ).ap()

    with tile.TileContext(nc) as tc:
        with (
            tc.tile_pool(name="const", bufs=1) as const,
            tc.tile_pool(name="xpool", bufs=1) as xpool,
            tc.tile_pool(name="psum", bufs=2, space="PSUM") as psum,
            tc.tile_pool(name="osb", bufs=4) as osb,
        ):
            xs = [
                xpool.tile([C, HP, WP], BF16, name=f"xs{n}", tag=f"xs{n}")
                for n in range(NPC)
            ]
            # ALL data DMAs ride the sync engine's ring: sync has no other
            # work, so a trigger blocking on a congested ring never stalls
            # compute (input triggers on the ACT engine were observed to
            # block its copy/ReLU stream mid-kernel).  fj=0 weight half
            # first -- the first matmul group only needs that half.
            nc.sync.dma_start(out=xs[0][:, 0:9, :], in_=xt[0, :, 0:9, :])
            w_sb = const.tile([C, 2, 8, F // 2], BF16, name="w_sb")
            nc.sync.dma_start(out=w_sb[:, 0], in_=wt[:, 0])
            nc.sync.dma_start(out=w_sb[:, 1], in_=wt[:, 1])
            b_sb = const.tile([F // 2, 2], F32, name="b_sb")
            nc.sync.dma_start(out=b_sb[:], in_=bs[:])
            # 16-row chunks: finer-grained arrival and twice the ring
            # round-robin share for input vs the queued output DMAs
            for n in range(NPC):
                bounds = (
                    [9, 25, 41, 57, 73, 89, 105, 121, 129] if n == 0
                    else [0, 17, 33, 49, 65, 81, 97, 113, 129]
                )
                for h0, h1 in zip(bounds[:-1], bounds[1:]):
                    nc.sync.dma_start(
                        out=xs[n][:, h0:h1, :], in_=xt[n, :, h0:h1, :]
                    )

            # Warm the PE HAM clock-gate during the input-DMA wait: HAM needs
            # ~3.4us of sustained PE-busy to lift the 1.2->2.4GHz throttle and
            # the PE program load only finishes ~4.9us in, so fill the window
            # until real data arrives (~7.5us) with one accumulating chain of
            # N=256 dummy matmuls (fine-grained so the handoff to real work
            # blocks by at most ~110ns).
            warm_sb = const.tile([128, 512], BF16, name="warm_sb")
            nc.vector.memset(warm_sb[:], 0.0)
            ps_w = psum.tile([128, 512], F32, name="ps_w", tag="ps_c")
            NWARM = 31
            for i in range(NWARM):
                nc.tensor.matmul(
                    ps_w[:, 0:128], warm_sb[:, 0:128], warm_sb[:, 0:128],
                    start=(i == 0), stop=(i == NWARM - 1))

            add = mybir.AluOpType.add
            # per-(image, fchunk) seam planes: seam[p] = k22 . x[2p+2, w=0]
            # (the c[-1] column every odd row needs at w=0).  The x column
            # rides in w_sb[:, n, 7]; all four planes are computed by tiny
            # N=64 matmuls right after warmup, gated only on the weight DMAs.
            seam_sb = [
                [
                    const.tile([128, 64, 1], F32, name=f"seam{n}{fj}")
                    for fj in range(2)
                ]
                for n in range(NPC)
            ]
            for n, fj in ((0, 0), (1, 0), (0, 1), (1, 1)):
                sp = psum.tile([128, 64, 1], F32, name="sps", tag="ps_c")
                nc.tensor.matmul(
                    sp[:], w_sb[:, fj, 6], w_sb[:, n, 7, 0:64],
                    start=True, stop=True)
                nc.vector.tensor_copy(seam_sb[n][fj][:], sp[:])
            ngrp = NPC * NHB * 2
            gi = 0
            for n in range(NPC):
                for hb in range(NHB):
                    r0 = 2 * HB * hb
                    rE = slice(r0, r0 + 2 * HB - 1, 2)        # rows 2i
                    rO = slice(r0 + 1, r0 + 2 * HB, 2)        # rows 2i+1
                    rC = slice(r0 + 2, r0 + 2 * HB + 1, 2)    # rows 2i+2
                    for fj in range(2):
                        gi += 1
                        last2 = gi > ngrp - 2

                        # Three PSUM accumulation groups, matmuls interleaved
                        # weight-major so identical stationary weights are
                        # back-to-back and ps_c finishes early (DVE pipeline).
                        # c[2i+2][w] = k21.x[w] + k22.x[w+1], w = 0..127
                        ps_c = psum.tile([128, HB, W], F32, name="ps_c", tag="ps_c")
                        # even: a[2i] + bb[2i+1]
                        ps_e = psum.tile(
                            [128, HB, W], F32, name="ps_e", tag="ps_e", bufs=3
                        )
                        # odd: a'[2i+1]; w=0 seam adds c[-1] = k22.x[0]
                        ps_o = psum.tile(
                            [128, HB, W], F32, name="ps_o", tag="ps_o", bufs=3
                        )
                        mm = nc.tensor.matmul
                        wv = w_sb[:, fj]
                        mm(ps_c[:], wv[:, 5], xs[n][:, rC, 1:129],
                           start=True, stop=False)
                        mm(ps_c[:], wv[:, 6], xs[n][:, rC, 2:130],
                           start=False, stop=True)
                        mm(ps_o[:], wv[:, 0], xs[n][:, rO, 0:128],
                           start=True, stop=False)
                        mm(ps_o[:], wv[:, 1], xs[n][:, rO, 1:129],
                           start=False, stop=True)
                        mm(ps_e[:], wv[:, 0], xs[n][:, rE, 1:129],
                           start=True, stop=False)
                        mm(ps_e[:], wv[:, 1], xs[n][:, rE, 2:130],
                           start=False, stop=False)
                        mm(ps_e[:], wv[:, 2], xs[n][:, rO, 0:128],
                           start=False, stop=False)
                        mm(ps_e[:], wv[:, 3], xs[n][:, rO, 1:129],
                           start=False, stop=False)
                        mm(ps_e[:], wv[:, 4], xs[n][:, rO, 2:130],
                           start=False, stop=True)

                        # DVE cannot read two PSUM operands in one op:
                        # stage c in SBUF, then add it into both parities.
                        # Alternate the copy engine to balance DVE vs ACT load
                        # (the last two groups pin it to ScalarE so the DVE
                        # adds start immediately after their matmuls).
                        # c staged with a SINGLE writer (a second writer to
                        # the same tile -- even a disjoint slice -- gets
                        # serialized by the dependency tracker and puts that
                        # engine's dispatch latency inside the ps_c-reuse
                        # chain).  Copy rides ScalarE 4 of 5 groups to
                        # balance DVE vs ACT (~1.66us each vs PE's 1.98us).
                        c_sb = osb.tile(
                            [128, HB, W], F32, name="c_sb", tag="cx", bufs=4
                        )
                        if last2 or (2 * hb + fj) % 5 != 0:
                            nc.scalar.copy(c_sb[:], ps_c[:])
                        else:
                            nc.vector.tensor_copy(c_sb[:], ps_c[:])
                        # both parities staged in one tile: [f, parity, row, w]
                        ob = osb.tile([128, 2, HB, W], F32, name="ob", tag="ob", bufs=4)
                        nc.vector.tensor_tensor(
                            ob[:, 1, :, 0:1], ps_o[:, :, 0:1],
                            seam_sb[n][fj][:, HB * hb : HB * hb + HB], op=add)
                        nc.vector.tensor_tensor(
                            ob[:, 1, :, 1:128], ps_o[:, :, 1:128],
                            c_sb[:, :, 0:127], op=add)
                        if gi != ngrp:
                            nc.vector.tensor_tensor(
                                ob[:, 0], ps_e[:], c_sb[:], op=add)

                        fo = osb.tile(
                            [128, 2, HB, W], BF16, name="fo", tag="fo", bufs=6
                        )
                        if last2:
                            # parity-split ReLU+DMA: the odd half (whose psum
                            # closes 5 matmuls earlier) drains while ps_e's
                            # matmuls are still streaming
                            nc.scalar.activation(
                                fo[:, 1], ob[:, 1],
                                mybir.ActivationFunctionType.Relu,
                                bias=b_sb[:, fj : fj + 1],
                            )
                            nc.sync.dma_start(
                                out=ot[n, hb, fsl_(fj), 1, :, :],
                                in_=fo[:, 1],
                            )
                            if gi == ngrp:
                                # final group: the even half (the only work
                                # fully exposed after the last matmul) drains
                                # in 2-row pieces so add/ReLU/DMA pipeline
                                for rh0 in (0, 2):
                                    rs = slice(rh0, rh0 + 2)
                                    nc.vector.tensor_tensor(
                                        ob[:, 0, rs], ps_e[:, rs],
                                        c_sb[:, rs], op=add)
                                    nc.scalar.activation(
                                        fo[:, 0, rs], ob[:, 0, rs],
                                        mybir.ActivationFunctionType.Relu,
                                        bias=b_sb[:, fj : fj + 1],
                                    )
                                    nc.sync.dma_start(
                                        out=ot[n, hb, fsl_(fj), 0, rs, :],
                                        in_=fo[:, 0, rs],
                                    )
                            else:
                                nc.scalar.activation(
                                    fo[:, 0], ob[:, 0],
                                    mybir.ActivationFunctionType.Relu,
                                    bias=b_sb[:, fj : fj + 1],
                                )
                                nc.sync.dma_start(
                                    out=ot[n, hb, fsl_(fj), 0, :, :],
                                    in_=fo[:, 0],
                                )
                        else:
                            nc.scalar.activation(
                                fo[:], ob[:],
                                mybir.ActivationFunctionType.Relu,
                                bias=b_sb[:, fj : fj + 1],
                            )
                            nc.sync.dma_start(
                                out=ot[n, hb, fsl_(fj), :, :, :], in_=fo[:]
                            )
    nc.compile()
    return nc


_NC_CACHE = None


def _get_nc():
    global _NC_CACHE
    if _NC_CACHE is None:
        _NC_CACHE = _build()
    return _NC_CACHE


def _prep_core_inputs(x_shard, wt_host, bs_host):
    xp = np.zeros((NPC, C, HP, WP), dtype=ml_dtypes.bfloat16)
    xp[:, :, :H, 1 : 1 + W] = x_shard.transpose(0, 3, 1, 2)
    wt = wt_host.copy()
    for n in range(NPC):
        # seam source: x[2p+2, w=0, c] for p=0..62 (p=63 is the zero pad row)
        wt[:, n, 7, 0:63] = (
            x_shard[n, 2:128:2, 0, :].T.astype(ml_dtypes.bfloat16)
        )
    return {"xt": xp, "wt": wt, "bs": bs_host}


def _unpack_out(ot_np):
    # ot: (NPC, NHB, F, 2, HB, W) bf16 -> (NPC, H, W, F) fp32
    # h = 8*hb + 2*i + par
    o = ot_np.astype(np.float32).transpose(0, 1, 4, 3, 5, 2)
    return o.reshape(NPC, H, W, F)


def _prep_host_weights(kernel, bias):
    # (C, 7, F) -> (C, fj, tap, 128), plus a spare tap slot 7 that each
    # core fills with its images' x[2p+2, w=0] columns (seam source)
    wt_host = np.zeros((C, 2, 8, F // 2), dtype=ml_dtypes.bfloat16)
    wt_host[:, :, 0:7, :] = (
        np.stack([kernel[r, c] for (r, c) in TAP_RC], axis=1)
        .reshape(C, 7, 2, F // 2)
        .transpose(0, 2, 1, 3)
    ).astype(ml_dtypes.bfloat16)
    bs_host = np.ascontiguousarray(
        bias.reshape(2, F // 2).T
    ).astype(np.float32)  # (128, 2): bs[f, j] = bias[j*128+f]
    return wt_host, bs_host


def kernel(x, kernel, bias):
    x = np.asarray(x, dtype=np.float32)
    kernel = np.asarray(kernel, dtype=np.float32)
    bias = np.asarray(bias, dtype=np.float32)

    wt_host, bs_host = _prep_host_weights(kernel, bias)

    nc = _get_nc()
    in_maps = [
        _prep_core_inputs(x[i * NPC : (i + 1) * NPC], wt_host, bs_host)
        for i in range(N_CORES)
    ]
    res = run_bass_kernel_spmd(nc, in_maps, list(range(N_CORES)))

    outs = [_unpack_out(res.results[i]["ot"]) for i in range(N_CORES)]
    return np.ascontiguousarray(np.concatenate(outs, axis=0))



# revision 17
# speedup vs baseline: 1.0143x; 1.0111x over previous
"""Hexagonal conv2d (HConv2D) Trainium2 kernel.

Math (verified vs the jax reference):
  out[n, 2i,   w, f] = relu(b + a[2i] + bb[2i+1] + c[2i+2])        (w-aligned)
  out[n, 2i+1, w, f] = relu(b + a'[2i+1][w] + c[2i+2][w-1])
with per-input-row 1D convs over Cin=128 -> F=256:
  a[r][w]  = k01.x[r,w]   + k02.x[r,w+1]
  a'[r][w] = k01.x[r,w-1] + k02.x[r,w]
  bb[r][w] = k10.x[r,w-1] + k11.x[r,w] + k12.x[r,w+1]
  c[r][w]  = k21.x[r,w]   + k22.x[r,w+1]
where krc = kernel[r, c] : [Cin, F].  KEY: c[2i+2] is shared between the
even and odd output rows (odd reads it shifted by one column), so it is
computed ONCE on the PE and added into both outputs on the Vector
engine: 9 big tap-matmuls per output-row-pair instead of 11 (18% PE cut).
The odd-row w=0 seam values (c[-1] = k22.x[2i+2, 0]) are computed up
front by four tiny N=64 matmuls whose x-column source rides inside the
weight DMA (spare tap slot 7), so no mid-stream op ever waits on them.

Distribution: data-parallel over batch (16 -> 8 cores x 2 images). Host
transposes x to [n, c, h, w] (c on partitions = contraction dim), pads
h/w with zeros, casts to bf16.  All data DMAs ride the sync engine's
HW-DGE ring (sync has no compute, so a trigger blocking on a congested
ring never stalls an engine that has real work; input triggers on the
ACT engine were observed to block its copy/ReLU stream mid-kernel).
The weight tensor is pre-split per f-chunk so the first matmul group
only waits for half of it.  A chain of N=128 dummy matmuls warms the PE
HAM clock-gate (1.2 -> 2.4 GHz needs ~3.4us of sustained PE-busy)
during the input-DMA wait.

PSUM tiles are [Fchunk=128, 4 rows, 128 w] (one bank, N=512; TRN2 PSUM
accumulation is fp32-only, so N=1024-bf16 is not available).  Per group
c is staged to SBUF by a SINGLE-writer copy (a second writer to the
same tile, even a disjoint slice, is serialized by the dependency
tracker into the ps_c-reuse chain); the odd parity adds the seam via a
tiny extra DVE op (odd emitted first - its psum closes 5 matmuls
earlier); one fused bias+ReLU on ScalarE emits bf16 and one DMA
per group writes a (n, hb, f, parity, row, w) bf16 DRAM layout
(per-partition contiguous 2 KB packets, half the bytes and packets of
fp32 NHWC).  The last two groups split ReLU+DMA by parity to shorten
the serial drain after the final matmul.  Host reassembles NHWC fp32.

Input arrives as 16-row chunks so the input stream holds twice the
DMA-ring round-robin share vs the queued output DMAs (input used to
land just-in-time at chunk boundaries, stalling the PE ~1us each).
Measured: ~145.8-147.1us on HW (baseline 147.5-148.6us), rel err
2.9e-3, 99% tensor-engine occupancy in the main phase; the remainder
is the bf16 PE feed floor (~130us: 294,912 moving columns at 1
col/cycle @ 2.4GHz + ~5ns/matmul issue) plus ~12us of fixed runtime
startup/teardown (engine preambles + ~9.8us semaphore-teardown
epilogue).
"""

import numpy as np
import ml_dtypes

import concourse.bacc as bacc
import concourse.bass as bass
import concourse.mybir as mybir
import concourse.tile as tile
from concourse.bass_utils import run_bass_kernel_spmd

N_CORES = 8
NPC = 2            # images per core
H = W = 128
C = 128            # input channels
F = 256            # filters
HP, WP = H + 1, W + 2
HB = 4             # out-row-pairs per psum tile (4 pairs -> N=512)
NHB = (H // 2) // HB

# tap weight order: kernel[r][c] for these (r, c)
TAP_RC = [(0, 1), (0, 2), (1, 0), (1, 1), (1, 2), (2, 1), (2, 2)]

BF16 = mybir.dt.bfloat16
F32 = mybir.dt.float32


def fsl_(fj):
    return slice(fj * 128, (fj + 1) * 128)


def _build():
    nc = bacc.Bacc(
        "TRN2", target_bir_lowering=False, debug=False, num_devices=N_CORES
    )
    xt = nc.dram_tensor("xt", (NPC, C, HP, WP), BF16, kind="ExternalInput").ap()
    # weights pre-split per f-chunk: (C, fj, tap, 128); tap slot 7 of half j
    # carries image j's x[2p+2, w=0] column (seam source), so the seam data
    # rides the weight DMA for free
    wt = nc.dram_tensor("wt", (C, 2, 8, F // 2), BF16, kind="ExternalInput").ap()
    bs = nc.dram_tensor("bs", (F // 2, 2), F32, kind="ExternalInput").ap()
    ot = nc.dram_tensor(
        "ot", (NPC, NHB, F, 2, HB, W), BF16, kind="ExternalOutput"
    ).ap()

    with tile.TileContext(nc) as tc:
        with (
            tc.tile_pool(name="const", bufs=1) as const,
            tc.tile_pool(name="xpool", bufs=1) as xpool,
            tc.tile_pool(name="psum", bufs=2, space="PSUM") as psum,
            tc.tile_pool(name="osb", bufs=4) as osb,
        ):
            xs = [
                xpool.tile([C, HP, WP], BF16, name=f"xs{n}", tag=f"xs{n}")
                for n in range(NPC)
            ]
            # ALL data DMAs ride the sync engine's ring: sync has no other
            # work, so a trigger blocking on a congested ring never stalls
            # compute (input triggers on the ACT engine were observed to
            # block its copy/ReLU stream mid-kernel).  fj=0 weight half
            # first -- the first matmul group only needs that half.
            nc.sync.dma_start(out=xs[0][:, 0:9, :], in_=xt[0, :, 0:9, :])
            w_sb = const.tile([C, 2, 8, F // 2], BF16, name="w_sb")
            nc.sync.dma_start(out=w_sb[:, 0], in_=wt[:, 0])
            nc.sync.dma_start(out=w_sb[:, 1], in_=wt[:, 1])
            b_sb = const.tile([F // 2, 2], F32, name="b_sb")
            nc.sync.dma_start(out=b_sb[:], in_=bs[:])
            # 16-row chunks: finer-grained arrival and twice the ring
            # round-robin share for input vs the queued output DMAs
            for n in range(NPC):
                bounds = (
                    [9, 25, 41, 57, 73, 89, 105, 121, 129] if n == 0
                    else [0, 17, 33, 49, 65, 81, 97, 113, 129]
                )
                for h0, h1 in zip(bounds[:-1], bounds[1:]):
                    nc.sync.dma_start(
                        out=xs[n][:, h0:h1, :], in_=xt[n, :, h0:h1, :]
                    )

            # Warm the PE HAM clock-gate during the input-DMA wait: HAM needs
            # ~3.4us of sustained PE-busy to lift the 1.2->2.4GHz throttle and
            # the PE program load only finishes ~4.9us in, so fill the window
            # until real data arrives (~7.5us) with one accumulating chain of
            # N=256 dummy matmuls (fine-grained so the handoff to real work
            # blocks by at most ~110ns).
            warm_sb = const.tile([128, 512], BF16, name="warm_sb")
            nc.vector.memset(warm_sb[:], 0.0)
            ps_w = psum.tile([128, 512], F32, name="ps_w", tag="ps_c")
            NWARM = 28
            for i in range(NWARM):
                nc.tensor.matmul(
                    ps_w[:, 0:128], warm_sb[:, 0:128], warm_sb[:, 0:128],
                    start=(i == 0), stop=(i == NWARM - 1))

            add = mybir.AluOpType.add
            # per-(image, fchunk) seam planes: seam[p] = k22 . x[2p+2, w=0]
            # (the c[-1] column every odd row needs at w=0).  The x column
            # rides in w_sb[:, n, 7]; all four planes are computed by tiny
            # N=64 matmuls right after warmup, gated only on the weight DMAs.
            seam_sb = [
                [
                    const.tile([128, 64, 1], F32, name=f"seam{n}{fj}")
                    for fj in range(2)
                ]
                for n in range(NPC)
            ]
            for n, fj in ((0, 0), (1, 0), (0, 1), (1, 1)):
                sp = psum.tile([128, 64, 1], F32, name="sps", tag="ps_c")
                nc.tensor.matmul(
                    sp[:], w_sb[:, fj, 6], w_sb[:, n, 7, 0:64],
                    start=True, stop=True)
                nc.vector.tensor_copy(seam_sb[n][fj][:], sp[:])
            ngrp = NPC * NHB * 2
            gi = 0
            for n in range(NPC):
                for hb in range(NHB):
                    r0 = 2 * HB * hb
                    rE = slice(r0, r0 + 2 * HB - 1, 2)        # rows 2i
                    rO = slice(r0 + 1, r0 + 2 * HB, 2)        # rows 2i+1
                    rC = slice(r0 + 2, r0 + 2 * HB + 1, 2)    # rows 2i+2
                    for fj in range(2):
                        gi += 1
                        last2 = gi > ngrp - 2

                        # Three PSUM accumulation groups, matmuls interleaved
                        # weight-major so identical stationary weights are
                        # back-to-back and ps_c finishes early (DVE pipeline).
                        # c[2i+2][w] = k21.x[w] + k22.x[w+1], w = 0..127
                        ps_c = psum.tile([128, HB, W], F32, name="ps_c", tag="ps_c")
                        # even: a[2i] + bb[2i+1]
                        ps_e = psum.tile(
                            [128, HB, W], F32, name="ps_e", tag="ps_e", bufs=3
                        )
                        # odd: a'[2i+1]; w=0 seam adds c[-1] = k22.x[0]
                        ps_o = psum.tile(
                            [128, HB, W], F32, name="ps_o", tag="ps_o", bufs=3
                        )
                        mm = nc.tensor.matmul
                        wv = w_sb[:, fj]
                        mm(ps_c[:], wv[:, 5], xs[n][:, rC, 1:129],
                           start=True, stop=False)
                        mm(ps_c[:], wv[:, 6], xs[n][:, rC, 2:130],
                           start=False, stop=True)
                        mm(ps_o[:], wv[:, 0], xs[n][:, rO, 0:128],
                           start=True, stop=False)
                        mm(ps_o[:], wv[:, 1], xs[n][:, rO, 1:129],
                           start=False, stop=True)
                        mm(ps_e[:], wv[:, 0], xs[n][:, rE, 1:129],
                           start=True, stop=False)
                        mm(ps_e[:], wv[:, 1], xs[n][:, rE, 2:130],
                           start=False, stop=False)
                        mm(ps_e[:], wv[:, 2], xs[n][:, rO, 0:128],
                           start=False, stop=False)
                        mm(ps_e[:], wv[:, 3], xs[n][:, rO, 1:129],
                           start=False, stop=False)
                        mm(ps_e[:], wv[:, 4], xs[n][:, rO, 2:130],
                           start=False, stop=True)

                        # DVE cannot read two PSUM operands in one op:
                        # stage c in SBUF, then add it into both parities.
                        # Alternate the copy engine to balance DVE vs ACT load
                        # (the last two groups pin it to ScalarE so the DVE
                        # adds start immediately after their matmuls).
                        # c staged with a SINGLE writer (a second writer to
                        # the same tile -- even a disjoint slice -- gets
                        # serialized by the dependency tracker and puts that
                        # engine's dispatch latency inside the ps_c-reuse
                        # chain).  Copy rides ScalarE 4 of 5 groups to
                        # balance DVE vs ACT (~1.66us each vs PE's 1.98us).
                        c_sb = osb.tile(
                            [128, HB, W], F32, name="c_sb", tag="cx", bufs=4
                        )
                        if last2 or (2 * hb + fj) % 5 != 0:
                            nc.scalar.copy(c_sb[:], ps_c[:])
                        else:
                            nc.vector.tensor_copy(c_sb[:], ps_c[:])
                        # both parities staged in one tile: [f, parity, row, w]
                        ob = osb.tile([128, 2, HB, W], F32, name="ob", tag="ob", bufs=5)
                        nc.vector.tensor_tensor(
                            ob[:, 1, :, 0:1], ps_o[:, :, 0:1],
                            seam_sb[n][fj][:, HB * hb : HB * hb + HB], op=add)
                        nc.vector.tensor_tensor(
                            ob[:, 1, :, 1:128], ps_o[:, :, 1:128],
                            c_sb[:, :, 0:127], op=add)
                        nc.vector.tensor_tensor(
                            ob[:, 0], ps_e[:], c_sb[:], op=add)

                        fo = osb.tile(
                            [128, 2, HB, W], BF16, name="fo", tag="fo", bufs=8
                        )
                        if last2:
                            # parity-split ReLU+DMA: the even half drains
                            # while the odd adds are still running
                            for par in (1, 0):
                                nc.scalar.activation(
                                    fo[:, par], ob[:, par],
                                    mybir.ActivationFunctionType.Relu,
                                    bias=b_sb[:, fj : fj + 1],
                                )
                                nc.sync.dma_start(
                                    out=ot[n, hb, fsl_(fj), par, :, :],
                                    in_=fo[:, par],
                                )
                        else:
                            nc.scalar.activation(
                                fo[:], ob[:],
                                mybir.ActivationFunctionType.Relu,
                                bias=b_sb[:, fj : fj + 1],
                            )
                            nc.sync.dma_start(
                                out=ot[n, hb, fsl_(fj), :, :, :], in_=fo[:]
                            )
    nc.compile()
    return nc


_NC_CACHE = None


def _get_nc():
    global _NC_CACHE
    if _NC_CACHE is None:
        _NC_CACHE = _build()
    return _NC_CACHE


def _prep_core_inputs(x_shard, wt_host, bs_host):
    xp = np.zeros((NPC, C, HP, WP), dtype=ml_dtypes.bfloat16)
    xp[:, :, :H, 1 : 1 + W] = x_shard.transpose(0, 3, 1, 2)
    wt = wt_host.copy()
    for n in range(NPC):
        # seam source: x[2p+2, w=0, c] for p=0..62 (p=63 is the zero pad row)
        wt[:, n, 7, 0:63] = (
            x_shard[n, 2:128:2, 0, :].T.astype(ml_dtypes.bfloat16)
        )
    return {"xt": xp, "wt": wt, "bs": bs_host}


def _unpack_out(ot_np):
    # ot: (NPC, NHB, F, 2, HB, W) bf16 -> (NPC, H, W, F) fp32
    # h = 8*hb + 2*i + par
    o = ot_np.astype(np.float32).transpose(0, 1, 4, 3, 5, 2)
    return o.reshape(NPC, H, W, F)


def _prep_host_weights(kernel, bias):
    # (C, 7, F) -> (C, fj, tap, 128), plus a spare tap slot 7 that each
    # core fills with its images' x[2p+2, w=0] columns (seam source)
    wt_host = np.zeros((C, 2, 8, F // 2), dtype=ml_dtypes.bfloat16)
    wt_host[:, :, 0:7, :] = (
        np.stack([kernel[r, c] for (r, c) in TAP_RC], axis=1)
        .reshape(C, 7, 2, F // 2)
        .transpose(0, 2, 1, 3)
    ).astype(ml_dtypes.bfloat16)
    bs_host = np.ascontiguousarray(
        bias.reshape(2, F // 2).T
    ).astype(np.float32)  # (128, 2): bs[f, j] = bias[j*128+f]
    return wt_host, bs_host


def kernel(x, kernel, bias):
    x = np.asarray(x, dtype=np.float32)
    kernel = np.asarray(kernel, dtype=np.float32)
    bias = np.asarray(bias, dtype=np.float32)

    wt_host, bs_host = _prep_host_weights(kernel, bias)

    nc = _get_nc()
    in_maps = [
        _prep_core_inputs(x[i * NPC : (i + 1) * NPC], wt_host, bs_host)
        for i in range(N_CORES)
    ]
    res = run_bass_kernel_spmd(nc, in_maps, list(range(N_CORES)))

    outs = [_unpack_out(res.results[i]["ot"]) for i in range(N_CORES)]
    return np.ascontiguousarray(np.concatenate(outs, axis=0))



# revision 18
# speedup vs baseline: 1.0144x; 1.0002x over previous
"""Hexagonal conv2d (HConv2D) Trainium2 kernel.

Math (verified vs the jax reference):
  out[n, 2i,   w, f] = relu(b + a[2i] + bb[2i+1] + c[2i+2])        (w-aligned)
  out[n, 2i+1, w, f] = relu(b + a'[2i+1][w] + c[2i+2][w-1])
with per-input-row 1D convs over Cin=128 -> F=256:
  a[r][w]  = k01.x[r,w]   + k02.x[r,w+1]
  a'[r][w] = k01.x[r,w-1] + k02.x[r,w]
  bb[r][w] = k10.x[r,w-1] + k11.x[r,w] + k12.x[r,w+1]
  c[r][w]  = k21.x[r,w]   + k22.x[r,w+1]
where krc = kernel[r, c] : [Cin, F].  KEY: c[2i+2] is shared between the
even and odd output rows (odd reads it shifted by one column), so it is
computed ONCE on the PE and added into both outputs on the Vector
engine: 9 big tap-matmuls per output-row-pair instead of 11 (18% PE cut).
The odd-row w=0 seam values (c[-1] = k22.x[2i+2, 0]) are computed up
front by four tiny N=64 matmuls whose x-column source rides inside the
weight DMA (spare tap slot 7), so no mid-stream op ever waits on them.

Distribution: data-parallel over batch (16 -> 8 cores x 2 images). Host
transposes x to [n, c, h, w] (c on partitions = contraction dim), pads
h/w with zeros, casts to bf16.  All data DMAs ride the sync engine's
HW-DGE ring (sync has no compute, so a trigger blocking on a congested
ring never stalls an engine that has real work; input triggers on the
ACT engine were observed to block its copy/ReLU stream mid-kernel).
The weight tensor is pre-split per f-chunk so the first matmul group
only waits for half of it.  A chain of N=128 dummy matmuls warms the PE
HAM clock-gate (1.2 -> 2.4 GHz needs ~3.4us of sustained PE-busy)
during the input-DMA wait.

PSUM tiles are [Fchunk=128, 4 rows, 128 w] (one bank, N=512; TRN2 PSUM
accumulation is fp32-only, so N=1024-bf16 is not available).  Per group
c is staged to SBUF by a SINGLE-writer copy (a second writer to the
same tile, even a disjoint slice, is serialized by the dependency
tracker into the ps_c-reuse chain); the odd parity adds the seam via a
tiny extra DVE op (odd emitted first - its psum closes 5 matmuls
earlier); one fused bias+ReLU on ScalarE emits bf16 and one DMA
per group writes a (n, hb, f, parity, row, w) bf16 DRAM layout
(per-partition contiguous 2 KB packets, half the bytes and packets of
fp32 NHWC).  The last two groups split ReLU+DMA by parity to shorten
the serial drain after the final matmul.  Host reassembles NHWC fp32.

Input arrives as 16-row chunks so the input stream holds twice the
DMA-ring round-robin share vs the queued output DMAs (input used to
land just-in-time at chunk boundaries, stalling the PE ~1us each).
Measured: ~145.8-147.1us on HW (baseline 147.5-148.6us), rel err
2.9e-3, 99% tensor-engine occupancy in the main phase; the remainder
is the bf16 PE feed floor (~130us: 294,912 moving columns at 1
col/cycle @ 2.4GHz + ~5ns/matmul issue) plus ~12us of fixed runtime
startup/teardown (engine preambles + ~9.8us semaphore-teardown
epilogue).
"""

import numpy as np
import ml_dtypes

import concourse.bacc as bacc
import concourse.bass as bass
import concourse.mybir as mybir
import concourse.tile as tile
from concourse.bass_utils import run_bass_kernel_spmd

N_CORES = 8
NPC = 2            # images per core
H = W = 128
C = 128            # input channels
F = 256            # filters
HP, WP = H + 1, W + 2
HB = 4             # out-row-pairs per psum tile (4 pairs -> N=512)
NHB = (H // 2) // HB

# tap weight order: kernel[r][c] for these (r, c)
TAP_RC = [(0, 1), (0, 2), (1, 0), (1, 1), (1, 2), (2, 1), (2, 2)]

BF16 = mybir.dt.bfloat16
F32 = mybir.dt.float32


def fsl_(fj):
    return slice(fj * 128, (fj + 1) * 128)


def _build():
    nc = bacc.Bacc(
        "TRN2", target_bir_lowering=False, debug=False, num_devices=N_CORES
    )
    xt = nc.dram_tensor("xt", (NPC, C, HP, WP), BF16, kind="ExternalInput").ap()
    # weights pre-split per f-chunk: (C, fj, tap, 128); tap slot 7 of half j
    # carries image j's x[2p+2, w=0] column (seam source), so the seam data
    # rides the weight DMA for free
    wt = nc.dram_tensor("wt", (C, 2, 8, F // 2), BF16, kind="ExternalInput").ap()
    bs = nc.dram_tensor("bs", (F // 2, 2), F32, kind="ExternalInput").ap()
    ot = nc.dram_tensor(
        "ot", (NPC, NHB, F, 2, HB, W), BF16, kind="ExternalOutput"
    ).ap()

    with tile.TileContext(nc) as tc:
        with (
            tc.tile_pool(name="const", bufs=1) as const,
            tc.tile_pool(name="xpool", bufs=1) as xpool,
            tc.tile_pool(name="psum", bufs=2, space="PSUM") as psum,
            tc.tile_pool(name="osb", bufs=4) as osb,
        ):
            xs = [
                xpool.tile([C, HP, WP], BF16, name=f"xs{n}", tag=f"xs{n}")
                for n in range(NPC)
            ]
            # ALL data DMAs ride the sync engine's ring: sync has no other
            # work, so a trigger blocking on a congested ring never stalls
            # compute (input triggers on the ACT engine were observed to
            # block its copy/ReLU stream mid-kernel).  fj=0 weight half
            # first -- the first matmul group only needs that half.
            nc.sync.dma_start(out=xs[0][:, 0:9, :], in_=xt[0, :, 0:9, :])
            w_sb = const.tile([C, 2, 8, F // 2], BF16, name="w_sb")
            nc.sync.dma_start(out=w_sb[:, 0], in_=wt[:, 0])
            nc.sync.dma_start(out=w_sb[:, 1], in_=wt[:, 1])
            b_sb = const.tile([F // 2, 2], F32, name="b_sb")
            nc.sync.dma_start(out=b_sb[:], in_=bs[:])
            # 16-row chunks: finer-grained arrival and twice the ring
            # round-robin share for input vs the queued output DMAs
            for n in range(NPC):
                bounds = (
                    [9, 25, 41, 57, 73, 89, 105, 121, 129] if n == 0
                    else [0, 17, 33, 49, 65, 81, 97, 113, 129]
                )
                for h0, h1 in zip(bounds[:-1], bounds[1:]):
                    nc.sync.dma_start(
                        out=xs[n][:, h0:h1, :], in_=xt[n, :, h0:h1, :]
                    )

            # Warm the PE HAM clock-gate during the input-DMA wait: HAM needs
            # ~3.4us of sustained PE-busy to lift the 1.2->2.4GHz throttle and
            # the PE program load only finishes ~4.9us in, so fill the window
            # until real data arrives (~7.5us) with one accumulating chain of
            # N=256 dummy matmuls (fine-grained so the handoff to real work
            # blocks by at most ~110ns).
            warm_sb = const.tile([128, 512], BF16, name="warm_sb")
            nc.vector.memset(warm_sb[:], 0.0)
            ps_w = psum.tile([128, 512], F32, name="ps_w", tag="ps_c")
            NWARM = 31
            for i in range(NWARM):
                nc.tensor.matmul(
                    ps_w[:, 0:128], warm_sb[:, 0:128], warm_sb[:, 0:128],
                    start=(i == 0), stop=(i == NWARM - 1))

            add = mybir.AluOpType.add
            # per-(image, fchunk) seam planes: seam[p] = k22 . x[2p+2, w=0]
            # (the c[-1] column every odd row needs at w=0).  The x column
            # rides in w_sb[:, n, 7]; all four planes are computed by tiny
            # N=64 matmuls right after warmup, gated only on the weight DMAs.
            seam_sb = [
                [
                    const.tile([128, 64, 1], F32, name=f"seam{n}{fj}")
                    for fj in range(2)
                ]
                for n in range(NPC)
            ]
            for n, fj in ((0, 0), (1, 0), (0, 1), (1, 1)):
                sp = psum.tile([128, 64, 1], F32, name="sps", tag="ps_c")
                nc.tensor.matmul(
                    sp[:], w_sb[:, fj, 6], w_sb[:, n, 7, 0:64],
                    start=True, stop=True)
                nc.vector.tensor_copy(seam_sb[n][fj][:], sp[:])
            ngrp = NPC * NHB * 2
            gi = 0
            for n in range(NPC):
                for hb in range(NHB):
                    r0 = 2 * HB * hb
                    rE = slice(r0, r0 + 2 * HB - 1, 2)        # rows 2i
                    rO = slice(r0 + 1, r0 + 2 * HB, 2)        # rows 2i+1
                    rC = slice(r0 + 2, r0 + 2 * HB + 1, 2)    # rows 2i+2
                    for fj in range(2):
                        gi += 1
                        last2 = gi > ngrp - 2

                        # Three PSUM accumulation groups, matmuls interleaved
                        # weight-major so identical stationary weights are
                        # back-to-back and ps_c finishes early (DVE pipeline).
                        # c[2i+2][w] = k21.x[w] + k22.x[w+1], w = 0..127
                        ps_c = psum.tile([128, HB, W], F32, name="ps_c", tag="ps_c")
                        # even: a[2i] + bb[2i+1]
                        ps_e = psum.tile(
                            [128, HB, W], F32, name="ps_e", tag="ps_e", bufs=3
                        )
                        # odd: a'[2i+1]; w=0 seam adds c[-1] = k22.x[0]
                        ps_o = psum.tile(
                            [128, HB, W], F32, name="ps_o", tag="ps_o", bufs=3
                        )
                        mm = nc.tensor.matmul
                        wv = w_sb[:, fj]
                        mm(ps_c[:], wv[:, 5], xs[n][:, rC, 1:129],
                           start=True, stop=False)
                        mm(ps_c[:], wv[:, 6], xs[n][:, rC, 2:130],
                           start=False, stop=True)
                        mm(ps_o[:], wv[:, 0], xs[n][:, rO, 0:128],
                           start=True, stop=False)
                        mm(ps_o[:], wv[:, 1], xs[n][:, rO, 1:129],
                           start=False, stop=True)
                        mm(ps_e[:], wv[:, 0], xs[n][:, rE, 1:129],
                           start=True, stop=False)
                        mm(ps_e[:], wv[:, 1], xs[n][:, rE, 2:130],
                           start=False, stop=False)
                        mm(ps_e[:], wv[:, 2], xs[n][:, rO, 0:128],
                           start=False, stop=False)
                        mm(ps_e[:], wv[:, 3], xs[n][:, rO, 1:129],
                           start=False, stop=False)
                        mm(ps_e[:], wv[:, 4], xs[n][:, rO, 2:130],
                           start=False, stop=True)

                        # DVE cannot read two PSUM operands in one op:
                        # stage c in SBUF, then add it into both parities.
                        # Alternate the copy engine to balance DVE vs ACT load
                        # (the last two groups pin it to ScalarE so the DVE
                        # adds start immediately after their matmuls).
                        # c staged with a SINGLE writer (a second writer to
                        # the same tile -- even a disjoint slice -- gets
                        # serialized by the dependency tracker and puts that
                        # engine's dispatch latency inside the ps_c-reuse
                        # chain).  Copy rides ScalarE 4 of 5 groups to
                        # balance DVE vs ACT (~1.66us each vs PE's 1.98us).
                        c_sb = osb.tile(
                            [128, HB, W], F32, name="c_sb", tag="cx", bufs=4
                        )
                        if last2 or (2 * hb + fj) % 5 != 0:
                            nc.scalar.copy(c_sb[:], ps_c[:])
                        else:
                            nc.vector.tensor_copy(c_sb[:], ps_c[:])
                        # both parities staged in one tile: [f, parity, row, w]
                        ob = osb.tile([128, 2, HB, W], F32, name="ob", tag="ob", bufs=5)
                        nc.vector.tensor_tensor(
                            ob[:, 1, :, 0:1], ps_o[:, :, 0:1],
                            seam_sb[n][fj][:, HB * hb : HB * hb + HB], op=add)
                        nc.vector.tensor_tensor(
                            ob[:, 1, :, 1:128], ps_o[:, :, 1:128],
                            c_sb[:, :, 0:127], op=add)
                        if gi != ngrp:
                            nc.vector.tensor_tensor(
                                ob[:, 0], ps_e[:], c_sb[:], op=add)

                        fo = osb.tile(
                            [128, 2, HB, W], BF16, name="fo", tag="fo", bufs=8
                        )
                        if last2:
                            # parity-split ReLU+DMA: the odd half (whose psum
                            # closes 5 matmuls earlier) drains while ps_e's
                            # matmuls are still streaming
                            nc.scalar.activation(
                                fo[:, 1], ob[:, 1],
                                mybir.ActivationFunctionType.Relu,
                                bias=b_sb[:, fj : fj + 1],
                            )
                            nc.sync.dma_start(
                                out=ot[n, hb, fsl_(fj), 1, :, :],
                                in_=fo[:, 1],
                            )
                            if gi == ngrp:
                                # final group: the even half (the only work
                                # fully exposed after the last matmul) drains
                                # in 2-row pieces so add/ReLU/DMA pipeline
                                for rh0 in (0, 2):
                                    rs = slice(rh0, rh0 + 2)
                                    nc.vector.tensor_tensor(
                                        ob[:, 0, rs], ps_e[:, rs],
                                        c_sb[:, rs], op=add)
                                    nc.scalar.activation(
                                        fo[:, 0, rs], ob[:, 0, rs],
                                        mybir.ActivationFunctionType.Relu,
                                        bias=b_sb[:, fj : fj + 1],
                                    )
                                    nc.sync.dma_start(
                                        out=ot[n, hb, fsl_(fj), 0, rs, :],
                                        in_=fo[:, 0, rs],
                                    )
                            else:
                                nc.scalar.activation(
                                    fo[:, 0], ob[:, 0],
                                    mybir.ActivationFunctionType.Relu,
                                    bias=b_sb[:, fj : fj + 1],
                                )
                                nc.sync.dma_start(
                                    out=ot[n, hb, fsl_(fj), 0, :, :],
                                    in_=fo[:, 0],
                                )
                        else:
                            nc.scalar.activation(
                                fo[:], ob[:],
                                mybir.ActivationFunctionType.Relu,
                                bias=b_sb[:, fj : fj + 1],
                            )
                            nc.sync.dma_start(
                                out=ot[n, hb, fsl_(fj), :, :, :], in_=fo[:]
                            )
    nc.compile()
    return nc


_NC_CACHE = None


def _get_nc():
    global _NC_CACHE
    if _NC_CACHE is None:
        _NC_CACHE = _build()
    return _NC_CACHE


def _prep_core_inputs(x_shard, wt_host, bs_host):
    xp = np.zeros((NPC, C, HP, WP), dtype=ml_dtypes.bfloat16)
    xp[:, :, :H, 1 : 1 + W] = x_shard.transpose(0, 3, 1, 2)
    wt = wt_host.copy()
    for n in range(NPC):
        # seam source: x[2p+2, w=0, c] for p=0..62 (p=63 is the zero pad row)
        wt[:, n, 7, 0:63] = (
            x_shard[n, 2:128:2, 0, :].T.astype(ml_dtypes.bfloat16)
        )
    return {"xt": xp, "wt": wt, "bs": bs_host}


def _unpack_out(ot_np):
    # ot: (NPC, NHB, F, 2, HB, W) bf16 -> (NPC, H, W, F) fp32
    # h = 8*hb + 2*i + par
    o = ot_np.astype(np.float32).transpose(0, 1, 4, 3, 5, 2)
    return o.reshape(NPC, H, W, F)


def _prep_host_weights(kernel, bias):
    # (C, 7, F) -> (C, fj, tap, 128), plus a spare tap slot 7 that each
    # core fills with its images' x[2p+2, w=0] columns (seam source)
    wt_host = np.zeros((C, 2, 8, F // 2), dtype=ml_dtypes.bfloat16)
    wt_host[:, :, 0:7, :] = (
        np.stack([kernel[r, c] for (r, c) in TAP_RC], axis=1)
        .reshape(C, 7, 2, F // 2)
        .transpose(0, 2, 1, 3)
    ).astype(ml_dtypes.bfloat16)
    bs_host = np.ascontiguousarray(
        bias.reshape(2, F // 2).T
    ).astype(np.float32)  # (128, 2): bs[f, j] = bias[j*128+f]
    return wt_host, bs_host


def kernel(x, kernel, bias):
    x = np.asarray(x, dtype=np.float32)
    kernel = np.asarray(kernel, dtype=np.float32)
    bias = np.asarray(bias, dtype=np.float32)

    wt_host, bs_host = _prep_host_weights(kernel, bias)

    nc = _get_nc()
    in_maps = [
        _prep_core_inputs(x[i * NPC : (i + 1) * NPC], wt_host, bs_host)
        for i in range(N_CORES)
    ]
    res = run_bass_kernel_spmd(nc, in_maps, list(range(N_CORES)))

    outs = [_unpack_out(res.results[i]["ot"]) for i in range(N_CORES)]
    return np.ascontiguousarray(np.concatenate(outs, axis=0))

